# revision 9
# baseline (speedup 1.0000x reference)
"""Dense-MoE (all experts, softmax-gated) Trainium2 kernel.

Math reformulation (per token t):
  s1    = x @ [Wd_cat | Wg]                # one K=768 matmul -> [64 h1 | 8 logits]
  h1b   = s1[:64] + bd_cat
  exp_e = exp(s1[64:72] + bg)              # unnormalized gate
  h2    = h1b @ blockdiag(Wm) + bm_cat     # one K=64 matmul
  g64   = expand(exp)                      # K=8 matmul vs 0/1 matrix
  s3in  = [h2 * g64 ; exp]                 # [72]
  o     = s3in @ [[0, Wu_cat], [1, bu]]    # K=72 matmul; cols 0,1 = Z = sum_e exp_e
  out   = o[2:] / o[0]                     # softmax normalization folded to the end

Sharding: data-parallel over tokens, 8 cores, weights replicated.
"""

import numpy as np

B, S, D, E, R = 8, 4096, 768, 8, 8
NCORES = 8
T_CORE = B * S // NCORES          # 4096 tokens per core
TILE_T = 512                      # tokens per compute tile
N_TILES = T_CORE // TILE_T        # 8
EW = E * R                        # 64
KW = EW + E                       # 72
KC = D // 128                     # 6 contraction chunks for stage 1
JC = TILE_T // 128                # 4 token chunks of 128 per tile

MM_DT = "float32r"                # matmul compute dtype

_CACHE = {}


def _build_and_compile():
    """Build the Bass/Tile program once. Returns compiled nc."""
    from contextlib import ExitStack

    import concourse.bass as bass
    import concourse.tile as tile
    from concourse import bacc, mybir

    f32 = mybir.dt.float32
    mmdt = getattr(mybir.dt, MM_DT)
    AF = mybir.ActivationFunctionType
    ALU = mybir.AluOpType

    nc = bacc.Bacc("TRN2", target_bir_lowering=False, debug=False, num_devices=NCORES)

    x_d = nc.dram_tensor("x", [T_CORE, D], mmdt, kind="ExternalInput").ap()
    w1_d = nc.dram_tensor("w1", [128, KC * KW], mmdt, kind="ExternalInput").ap()
    wm_d = nc.dram_tensor("wm", [EW, EW], mmdt, kind="ExternalInput").ap()
    e8_d = nc.dram_tensor("e8", [E, EW], mmdt, kind="ExternalInput").ap()
    w3_d = nc.dram_tensor("w3", [KW, 2 + D], mmdt, kind="ExternalInput").ap()
    id_d = nc.dram_tensor("ident", [128, 128], mmdt, kind="ExternalInput").ap()
    bd_d = nc.dram_tensor("bd", [EW, 1], f32, kind="ExternalInput").ap()
    bm_d = nc.dram_tensor("bm", [EW, 1], f32, kind="ExternalInput").ap()
    bg_d = nc.dram_tensor("bg", [E, 1], f32, kind="ExternalInput").ap()
    out_d = nc.dram_tensor("out", [T_CORE, D], f32, kind="ExternalOutput").ap()

    # [n_tile, 128, JC, 768] views: partition p of tile i holds tokens i*512 + j*128 + p
    x_v = x_d.rearrange("(i j p) d -> i p j d", j=JC, p=128)
    out_v = out_d.rearrange("(i j p) d -> i p j d", j=JC, p=128)

    with tile.TileContext(nc) as tc, ExitStack() as ctx:
        const = ctx.enter_context(tc.tile_pool(name="const", bufs=1))
        xin = ctx.enter_context(tc.tile_pool(name="xin", bufs=3))
        xts = ctx.enter_context(tc.tile_pool(name="xts", bufs=2))
        mid = ctx.enter_context(tc.tile_pool(name="mid", bufs=2))
        outp = ctx.enter_context(tc.tile_pool(name="outp", bufs=3))
        small = ctx.enter_context(tc.tile_pool(name="small", bufs=4))
        # PSUM budget (8 banks): xtp 2 + s2 1 + g64 1 + s1s3 2x2 = 8
        xtp = ctx.enter_context(tc.tile_pool(name="xtp", bufs=2, space="PSUM"))
        s2p = ctx.enter_context(tc.tile_pool(name="s2p", bufs=1, space="PSUM"))
        g64p = ctx.enter_context(tc.tile_pool(name="g64p", bufs=1, space="PSUM"))
        s13p = ctx.enter_context(tc.tile_pool(name="s13p", bufs=2, space="PSUM"))

        w1_sb = const.tile([128, KC * KW], mmdt, name="w1_sb")
        nc.sync.dma_start(w1_sb[:], w1_d)
        wm_sb = const.tile([EW, EW], mmdt, name="wm_sb")
        nc.sync.dma_start(wm_sb[:], wm_d)
        e8_sb = const.tile([KW, EW], mmdt, name="e8_sb")
        nc.sync.dma_start(e8_sb[0:E, :], e8_d)
        w3_sb = const.tile([KW, 2 + D], mmdt, name="w3_sb")
        nc.sync.dma_start(w3_sb[:], w3_d)
        id_sb = const.tile([128, 128], mmdt, name="id_sb")
        nc.sync.dma_start(id_sb[:], id_d)
        bd_sb = const.tile([EW, 1], f32, name="bd_sb")
        nc.sync.dma_start(bd_sb[:], bd_d)
        bm_sb = const.tile([EW, 1], f32, name="bm_sb")
        nc.sync.dma_start(bm_sb[:], bm_d)
        bg_sb = const.tile([E, 1], f32, name="bg_sb")
        nc.sync.dma_start(bg_sb[:], bg_d)

        for i in range(N_TILES):
            x_sb = xin.tile([128, JC * D], mmdt, name="x_sb", tag="x")
            nc.sync.dma_start(
                x_sb[:].rearrange("p (j d) -> p j d", j=JC), x_v[i, :, :, :]
            )

            # transpose x on PE: 6 chunks of [128d, 512t]; DVE moves psum->sbuf
            xt_sb = xts.tile([128, KC * TILE_T], mmdt, name="xt_sb", tag="xt")
            for c in range(KC):
                xt_ps = xtp.tile([128, TILE_T], mmdt, name="xt_ps", tag="xtp")
                for j in range(JC):
                    nc.tensor.transpose(
                        xt_ps[:, j * 128:(j + 1) * 128],
                        x_sb[:, j * D + c * 128: j * D + (c + 1) * 128],
                        id_sb[:],
                    )
                nc.vector.tensor_copy(
                    xt_sb[:, c * TILE_T:(c + 1) * TILE_T], xt_ps[:]
                )

            # stage 1: [72, 512] = W1.T @ x.T (accumulate over 6 K-chunks)
            s1 = s13p.tile([KW, TILE_T], f32, name="s1", tag="s13")
            for c in range(KC):
                nc.tensor.matmul(
                    s1[:],
                    w1_sb[:, c * KW:(c + 1) * KW],
                    xt_sb[:, c * TILE_T:(c + 1) * TILE_T],
                    start=(c == 0),
                    stop=(c == KC - 1),
                )

            # h1b = s1[0:64] + bd  (ACT), exp(logits + bg) -> s3in[64:72] (ACT)
            h1b = mid.tile([EW, TILE_T], mmdt, name="h1b", tag="h1b")
            nc.scalar.activation(h1b[:], s1[0:EW, :], AF.Identity, bias=bd_sb[:])
            s3in = mid.tile([KW, TILE_T], mmdt, name="s3in", tag="s3in")
            nc.scalar.activation(s3in[EW:KW, :], s1[EW:KW, :], AF.Exp, bias=bg_sb[:])
            exp_sb = mid.tile([E, TILE_T], mmdt, name="exp_sb", tag="exp")
            nc.scalar.activation(exp_sb[:], s1[EW:KW, :], AF.Exp, bias=bg_sb[:])

            # stage 2 + gate expansion (e8 weights live at partitions 64-71)
            s2 = s2p.tile([EW, TILE_T], f32, name="s2", tag="s2")
            nc.tensor.matmul(s2[:], wm_sb[:], h1b[:], start=True, stop=True)
            g64_ps = g64p.tile([EW, TILE_T], f32, name="g64_ps", tag="g64p")
            nc.tensor.matmul(
                g64_ps[:], e8_sb[0:E, :], exp_sb[:], start=True, stop=True
            )

            g64 = mid.tile([EW, TILE_T], f32, name="g64", tag="g64")
            nc.scalar.copy(g64[:], g64_ps[:])
            # s3in[0:64] = (h2 + bm) * g64
            nc.vector.scalar_tensor_tensor(
                s3in[0:EW, :], s2[:], bm_sb[:], g64[:],
                op0=ALU.add, op1=ALU.mult,
            )

            # stage 3: per 128-token chunk. PSUM tile [128, 1024] spans 2 banks:
            # matmul A -> cols 0:386 (outs 0-383 + Z,Z) in bank 0,
            # matmul B -> cols 512:896 (outs 384-767) in bank 1.
            out_sb = outp.tile([128, JC * D], f32, name="out_sb", tag="out")
            for j in range(JC):
                lhsT = s3in[:, j * 128:(j + 1) * 128]
                s3 = s13p.tile([128, 1024], f32, name="s3", tag="s13")
                nc.tensor.matmul(
                    s3[:, 0:386], lhsT, w3_sb[:, 0:386], start=True, stop=True
                )
                nc.tensor.matmul(
                    s3[:, 512:896], lhsT, w3_sb[:, 386:770], start=True, stop=True
                )
                rc = small.tile([128, 1], f32, name="rc", tag="rc")
                nc.vector.reciprocal(rc[:], s3[:, 384:385])
                nc.scalar.mul(
                    out_sb[:, j * D:(j + 1) * D].rearrange(
                        "p (a b) -> p a b", a=2
                    ),
                    s3[:].rearrange("p (a b) -> p a b", a=2)[:, :, 0:384],
                    rc[:],
                )

            nc.scalar.dma_start(
                out_v[i, :, :, :], out_sb[:].rearrange("p (j d) -> p j d", j=JC)
            )

    nc.compile()
    return nc


def _pack_host_inputs(Wd, bd, Wm, bm, Wu, bu, Wg, bg):
    """Repack the tiny weights into the on-chip layouts (host-side, ~100KB)."""
    f = np.float32
    W1 = np.concatenate(
        [np.ascontiguousarray(Wd.transpose(1, 0, 2)).reshape(D, EW), Wg], axis=1
    ).astype(f)                                   # [768, 72]
    w1p = np.ascontiguousarray(
        W1.reshape(KC, 128, KW).transpose(1, 0, 2)
    ).reshape(128, KC * KW)                       # [128, 432]; chunk c at cols c*72

    wmbd = np.zeros((EW, EW), f)
    for e in range(E):
        wmbd[e * R:(e + 1) * R, e * R:(e + 1) * R] = Wm[e]

    e8 = np.kron(np.eye(E, dtype=f), np.ones((1, R), f))   # [8, 64]

    wu = Wu.reshape(EW, D)
    w3e = np.zeros((KW, 2 + D), f)
    w3e[:EW, 0:384] = wu[:, 0:384]
    w3e[EW:, 0:384] = bu[:, 0:384]
    w3e[EW:, 384] = 1.0
    w3e[EW:, 385] = 1.0
    w3e[:EW, 386:770] = wu[:, 384:768]
    w3e[EW:, 386:770] = bu[:, 384:768]

    return {
        "w1": w1p,
        "wm": wmbd,
        "e8": e8,
        "w3": w3e,
        "ident": np.eye(128, dtype=f),
        "bd": np.ascontiguousarray(bd.reshape(EW, 1)).astype(f),
        "bm": np.ascontiguousarray(bm.reshape(EW, 1)).astype(f),
        "bg": np.ascontiguousarray(bg.reshape(E, 1)).astype(f),
    }


def _run(inputs, trace=False, **kw):
    from concourse import bass_utils

    if "nc" not in _CACHE:
        _CACHE["nc"] = _build_and_compile()
    nc = _CACHE["nc"]

    x = np.ascontiguousarray(np.asarray(inputs["x"], dtype=np.float32)).reshape(
        B * S, D
    )
    w = _pack_host_inputs(
        *(np.asarray(inputs[k], dtype=np.float32)
          for k in ["Wd", "bd", "Wm", "bm", "Wu", "bu", "Wg", "bg"])
    )
    in_maps = [
        {"x": np.ascontiguousarray(x[i * T_CORE:(i + 1) * T_CORE]), **w}
        for i in range(NCORES)
    ]
    res = bass_utils.run_bass_kernel_spmd(
        nc, in_maps, core_ids=list(range(NCORES)), trace=trace, **kw
    )
    out = np.concatenate(
        [res.results[i]["out"] for i in range(NCORES)], axis=0
    ).reshape(B, S, D)
    return out, res


def kernel(**inputs) -> np.ndarray:
    out, _ = _run(inputs)
    return out
